# revision 6
# baseline (speedup 1.0000x reference)
import sys

sys.path.insert(0, "/opt/trn_rl_repo")

import numpy as np

# nn_GaussianMixture: log-likelihood of N points under an M-component GMM.
# d_ij = -0.5 (x_i-c_j)^T S_j (x_i-c_j) + log coef_j ; out_i = logsumexp_j d_ij - thr.
# Rewritten as d_ij = sum_r W[r,j] * zz[i,r] over 153 quadratic features of the
# augmented, centered point z = [x-0.5, 1]: feature r = 17f+d is z_d * z_{(d+f)%17},
# f in [0,9), so each unordered pair appears exactly once (off-diag weights doubled).
# The host precomputes the fp16 feature tiles already transposed into the PE
# stationary layout; the device does, per 1024-point group: 16 matmuls (128+32
# contraction split), a segmented max, a broadcast bias-matmul that subtracts the
# per-point max inside PSUM, one exp, and per-block sums; log + max re-add at the
# kernel tail.
N, M, D = 500000, 128, 16
N_CORES = 8
GRP = 1024                      # points per group (8 blocks of 128)
GROUPS_PER_CORE = 62
NPC = GRP * GROUPS_PER_CORE     # 63488 points per core
SHIFT = 0.5

_F16 = None  # lazy mybir handle cache
_MODULE_CACHE = {}

# feature index tables (r = 17f + d  ->  z_d * z_e, e = (d+f) % 17)
_DIDX = np.array([d for f in range(9) for d in range(17)], np.int64)
_EIDX = np.array([(d + f) % 17 for f in range(9) for d in range(17)], np.int64)


def _build_weights(centers, covs_inv_sqrt, weights, threshold):
    """W [153, M] fp32 such that d_ij = sum_r W[r, j] zz[i, r]."""
    L = np.asarray(covs_inv_sqrt, np.float64)
    c = np.asarray(centers, np.float64) - SHIFT
    w = np.abs(np.asarray(weights, np.float64))
    thr = float(np.asarray(threshold).reshape(-1)[0])
    covs = np.einsum('jde,jfe->jdf', L, L)
    cp = w / (w.sum() + 1e-30)
    lc = np.log(cp + 1e-300) + 0.5 * np.linalg.slogdet(covs)[1]
    A = np.zeros((M, 17, 17))
    A[:, :16, :16] = -0.5 * covs
    Sc = np.einsum('jde,je->jd', covs, c)
    A[:, :16, 16] = 0.5 * Sc
    A[:, 16, :16] = 0.5 * Sc
    A[:, 16, 16] = -0.5 * np.einsum('jd,jd->j', c, Sc) + lc - thr
    W = A[:, _DIDX, _EIDX].T.copy()          # [153, M]
    W[17:] *= 2.0                            # f >= 1 rows cover each pair once
    return W.astype(np.float32)


def _host_prep(points):
    """Per-core fp16 transposed feature tiles.

    Returns (zzT0 [N_CORES, G, 128, GRP], zzT1 [N_CORES, G, 32, GRP])."""
    pts = np.asarray(points, np.float32)
    n = pts.shape[0]
    ntot = N_CORES * NPC
    z = np.empty((ntot, 17), np.float32)
    z[:n, :16] = pts - SHIFT
    z[n:, :16] = 0.0
    z[:, 16] = 1.0
    zz = (z[:, _DIDX] * z[:, _EIDX]).astype(np.float16)      # [ntot, 153]
    zz0 = zz[:, :128].reshape(N_CORES, GROUPS_PER_CORE, GRP, 128)
    zzT0 = np.ascontiguousarray(zz0.transpose(0, 1, 3, 2))
    zz1 = np.zeros((ntot, 32), np.float16)
    zz1[:, :25] = zz[:, 128:153]
    zz1 = zz1.reshape(N_CORES, GROUPS_PER_CORE, GRP, 32)
    zzT1 = np.ascontiguousarray(zz1.transpose(0, 1, 3, 2))
    return zzT0, zzT1


def build_module(n_groups=GROUPS_PER_CORE, macro=4):
    import concourse.bacc as bacc
    import concourse.mybir as mybir
    import concourse.tile as tile
    from contextlib import ExitStack

    f16, f32 = mybir.dt.float16, mybir.dt.float32
    nc = bacc.Bacc("TRN2", target_bir_lowering=False, debug=False,
                   enable_asserts=True, num_devices=N_CORES)
    zzT0 = nc.dram_tensor("zzT0", [n_groups, 128, GRP], f16, kind="ExternalInput").ap()
    zzT1 = nc.dram_tensor("zzT1", [n_groups, 32, GRP], f16, kind="ExternalInput").ap()
    w0 = nc.dram_tensor("W0", [128, 128], f16, kind="ExternalInput").ap()
    w1 = nc.dram_tensor("W1", [32, 128], f16, kind="ExternalInput").ap()
    msk = nc.dram_tensor("MASK", [8, GRP], f16, kind="ExternalInput").ap()
    id32 = nc.dram_tensor("ID32", [128, 128], f32, kind="ExternalInput").ap()
    out = nc.dram_tensor("out", [n_groups * GRP], f32, kind="ExternalOutput").ap()

    ncols = n_groups * 8    # one col of sacc/negmx per 128-pt block

    with tile.TileContext(nc) as tc, ExitStack() as ctx:
        cpool = ctx.enter_context(tc.tile_pool(name="consts", bufs=1))
        w0_t = cpool.tile([128, 128], f16, tag="w0")
        nc.sync.dma_start(w0_t[:], w0[:])
        w1_t = cpool.tile([32, 128], f16, tag="w1")
        nc.sync.dma_start(w1_t[:], w1[:])
        msk_t = cpool.tile([8, GRP], f16, tag="msk")
        nc.sync.dma_start(msk_t[:], msk[:])
        id32_t = cpool.tile([128, 128], f32, tag="id32")
        nc.sync.dma_start(id32_t[:], id32[:])
        # persistent accumulators across the whole kernel
        sacc = cpool.tile([128, ncols], f32, tag="sacc")
        nmxa = cpool.tile([128, ncols], f16, tag="nmxa")

        zpool = ctx.enter_context(tc.tile_pool(name="zz", bufs=3))
        dpool = ctx.enter_context(tc.tile_pool(name="d", bufs=3, space="PSUM"))
        epool = ctx.enter_context(tc.tile_pool(name="e", bufs=2))
        npool = ctx.enter_context(tc.tile_pool(name="nmx", bufs=2))
        ntpool = ctx.enter_context(tc.tile_pool(name="nmxT", bufs=2))
        spool = ctx.enter_context(tc.tile_pool(name="scr", bufs=2))
        fpool = ctx.enter_context(tc.tile_pool(name="fin", bufs=2))
        fppool = ctx.enter_context(tc.tile_pool(name="finp", bufs=2, space="PSUM"))

        for g in range(n_groups):
            z0 = zpool.tile([128, GRP], f16, tag="z0")
            nc.sync.dma_start(z0[:], zzT0[g])
            z1 = zpool.tile([32, GRP], f16, tag="z1")
            nc.sync.dma_start(z1[:], zzT1[g])
            d = dpool.tile([128, GRP], f32, tag="d")
            for b in range(8):
                sl = slice(b * 128, (b + 1) * 128)
                # start=True zeroes the whole 2KB PSUM bank: only on the
                # first matmul into each of the two banks (b==0 / b==4)
                nc.tensor.matmul(d[:, sl], z0[:, sl], w0_t[:],
                                 start=(b in (0, 4)), stop=False,
                                 skip_group_check=True)
                nc.tensor.matmul(d[:, sl], z1[:, sl], w1_t[:],
                                 start=False, stop=False, skip_group_check=True)
            # negmx[:, 0:8] = -max_j d over each 128-center block; rest zero-pad
            # so the xbar transpose has a full 128-col source.
            negmx = npool.tile([128, 128], f16, tag="negmx")
            nc.gpsimd.memset(negmx[:, 8:128], 0.0)
            nc.vector.tensor_reduce(
                negmx[:, 0:8], d[:].rearrange("p (b j) -> p b j", b=8),
                axis=mybir.AxisListType.X, op=mybir.AluOpType.max, negate=True)
            nc.gpsimd.tensor_copy(nmxa[:, g * 8:(g + 1) * 8], negmx[:, 0:8])
            negmxT = ntpool.tile([128, 128], f16, tag="negmxT")
            nc.sync.dma_start_transpose(negmxT[:], negmx[:])
            # d += negmx (broadcast per point via constant 0/1 mask);
            # two matmuls so each stays within one PSUM bank
            nc.tensor.matmul(d[:, 0:512], negmxT[0:8, :], msk_t[:, 0:512],
                             start=False, stop=True, skip_group_check=True)
            nc.tensor.matmul(d[:, 512:1024], negmxT[0:8, :], msk_t[:, 512:1024],
                             start=False, stop=True, skip_group_check=True)
            e = epool.tile([128, GRP], f16, tag="e")
            nc.scalar.activation(e[:], d[:], mybir.ActivationFunctionType.Exp)
            for b in range(8):
                scr = spool.tile([128, 128], f16, tag="scr")
                nc.vector.tensor_scalar(
                    scr[:], e[:, b * 128:(b + 1) * 128], 1.0, None,
                    op0=mybir.AluOpType.mult, op1=mybir.AluOpType.add,
                    accum_out=sacc[:, g * 8 + b:g * 8 + b + 1])

        # tail: out = -negmx... no: out = max + ln(s) = ln(s) - negmx
        lg = cpool.tile([128, ncols], f32, tag="lg")
        nc.scalar.activation(lg[:], sacc[:], mybir.ActivationFunctionType.Ln)
        fin = cpool.tile([128, ncols], f32, tag="fin")
        nc.vector.tensor_tensor(fin[:], lg[:], nmxa[:],
                                op=mybir.AluOpType.subtract)
        # transpose [128, ncols] -> [ncols, 128] in <=128-col pieces, DMA out
        csz = 124
        for s in range(0, ncols, csz):
            w = min(csz, ncols - s)
            fp = fppool.tile([w, 128], f32, tag="fp")
            nc.tensor.transpose(fp[:], fin[:, s:s + w], id32_t[:])
            ft = fpool.tile([w, 128], f32, tag="ft")
            nc.vector.tensor_copy(ft[:], fp[:])
            nc.sync.dma_start(
                out[s * 128:(s + w) * 128].rearrange("(a b) -> a b", b=128),
                ft[:])
    nc.compile()
    return nc


def _get_module(n_groups=GROUPS_PER_CORE):
    key = n_groups
    if key not in _MODULE_CACHE:
        _MODULE_CACHE[key] = build_module(n_groups)
    return _MODULE_CACHE[key]


def make_in_maps(points, centers, covs_inv_sqrt, weights, threshold):
    W = _build_weights(centers, covs_inv_sqrt, weights, threshold)
    W0 = W[:128].astype(np.float16)
    W1 = np.zeros((32, 128), np.float16)
    W1[:25] = W[128:153].astype(np.float16)
    mask = np.zeros((8, GRP), np.float16)
    for b in range(8):
        mask[b, b * 128:(b + 1) * 128] = 1.0
    id32 = np.eye(128, dtype=np.float32)
    zzT0, zzT1 = _host_prep(points)
    return [{"zzT0": zzT0[c], "zzT1": zzT1[c], "W0": W0, "W1": W1,
             "MASK": mask, "ID32": id32} for c in range(N_CORES)]


def kernel(points, centers, covs_inv_sqrt, weights, threshold):
    from concourse.bass_utils import run_bass_kernel_spmd
    from concourse.bass_interp import get_hw_module

    nc = _get_module()
    in_maps = make_in_maps(points, centers, covs_inv_sqrt, weights, threshold)
    old_m = nc.m
    nc.m = get_hw_module(nc.m)
    try:
        res = run_bass_kernel_spmd(nc, in_maps, list(range(N_CORES)))
    finally:
        nc.m = old_m
    full = np.concatenate([res.results[c]["out"] for c in range(N_CORES)])
    return full[:N].reshape(N, 1).astype(np.float32)
